# revision 10
# baseline (speedup 1.0000x reference)
"""Trainium2 Bass kernel for a single attention head with input projections.

Per-batch-element (B=8 -> one NeuronCore each):
  k = key @ Wk^T, q = query @ Wq^T, v = value @ Wv^T          [T, H]
  S = q @ k^T / sqrt(E); mask kidx <= qidx+1; P = softmax(S)
  out = P @ v                                                  [T, H]

T=2048, E=1024, H=2048.

Layout strategy: all matmuls contract over the partition dim, so the host
pre-transposes activations and weights to [E, T] / [E, H] (and casts to
bf16).  Scores are computed TRANSPOSED (S^T[tk, tq] = k-tiles as lhsT,
q-tiles as rhs) so that:
  - P^T tiles feed the P@V matmul directly as lhsT (no on-chip transpose),
  - the softmax denominator is a ones-vector matmul over the partition dim.
No max-subtraction is needed: |S| <= ~10 here, exp() is safe in fp32.
"""

import sys

sys.path.insert(0, "/opt/trn_rl_repo")

import ml_dtypes
import numpy as np

import concourse.bass as bass
import concourse.mybir as mybir
import concourse.tile as tile
from concourse import bass_utils
from concourse.tile import ScopedClock

B, T, E, H = 8, 2048, 1024, 2048
P = 128
EO = E // P          # 8 e-subtiles
HO = H // P          # 16 h-subtiles
TKT = T // P         # 16 tk tiles
NBLK = 4             # tq blocks of 512
BLK = T // NBLK      # 512
NMASK = 5            # distinct partial-mask patterns per tq block
BF16 = mybir.dt.bfloat16
F32 = mybir.dt.float32


class _SplitDrainTC(tile.TileContext):
    """This walrus build rejects >1 sync-wait on the kernel-tail SP Drain
    ("Too many sync wait commands").  Spread the waits over preceding nops
    on the same engine instead — sequentially equivalent."""

    def _drain_and_barrier(self, tick_clock, wait_clock):
        nc = self.nc
        nops = [nc.sync.nop(nofuse=True) for _ in range(40)]
        drain_inst = nc.sync.drain()
        wait_clock.add_sem_waits(
            drain_inst.ins, ScopedClock({None: tick_clock.global_clock})
        )
        si = drain_inst.ins.sync_info
        waits = list(si.on_wait or [])
        if len(waits) > 1:
            assert len(waits) <= len(nops) + 1
            si.on_wait = [waits[-1]]
            for w, nop in zip(waits[:-1], nops):
                nsi = nop.ins.sync_info
                if nsi is None:
                    nop.ins.sync_info = mybir.SyncInfo(on_wait=[w], on_update=[])
                else:
                    nsi.on_wait = [w]
        nc.all_engine_barrier()
        popped = nc._tile_sem_poison_stack.pop()
        assert popped is self._sem_poison
        nc.clear_and_free_semaphores(list(self.sems.allocated().values()))
        nc.all_engine_barrier()


def _build():
    nc = bass.Bass("TRN2", target_bir_lowering=False, debug=False)

    xq = nc.dram_tensor("xq", (E, T), BF16, kind="ExternalInput").ap()
    xk = nc.dram_tensor("xk", (E, T), BF16, kind="ExternalInput").ap()
    xv = nc.dram_tensor("xv", (E, T), BF16, kind="ExternalInput").ap()
    wq = nc.dram_tensor("wq", (E, H), BF16, kind="ExternalInput").ap()
    wk = nc.dram_tensor("wk", (E, H), BF16, kind="ExternalInput").ap()
    wv = nc.dram_tensor("wv", (E, H), BF16, kind="ExternalInput").ap()
    masks = nc.dram_tensor("masks", (P, 2 * BLK), BF16, kind="ExternalInput").ap()
    out = nc.dram_tensor("out", (T, H), F32, kind="ExternalOutput").ap()

    def et(a):  # [E, X] dram -> [128, EO, X] view
        return a.rearrange("(eo p) t -> p eo t", p=P)

    with _SplitDrainTC(nc) as tc:
        with (
            tc.tile_pool(name="wkv", bufs=1) as wkv_pool,
            tc.tile_pool(name="wqp", bufs=1) as wq_pool,
            tc.tile_pool(name="xblk", bufs=2) as x_pool,
            tc.tile_pool(name="ktres", bufs=1) as kt_pool,
            tc.tile_pool(name="qt", bufs=1) as qt_pool,
            tc.tile_pool(name="pt", bufs=1) as pt_pool,
            tc.tile_pool(name="vst", bufs=16) as v_pool,
            tc.tile_pool(name="vpj", bufs=1) as vproj_pool,
            tc.tile_pool(name="outs", bufs=2) as out_pool,
            tc.tile_pool(name="misc", bufs=1) as misc_pool,
            tc.tile_pool(name="ps_a", bufs=3, space="PSUM") as ps_a,
            tc.tile_pool(name="ps_o", bufs=3, space="PSUM") as ps_o,
            tc.tile_pool(name="ps_d", bufs=1, space="PSUM") as ps_d,
            tc.tile_pool(name="dram", bufs=1, space="DRAM") as dram_pool,
        ):
            masks_sb = misc_pool.tile([P, 2 * BLK], BF16, tag="masks")
            nc.sync.dma_start(masks_sb[:], masks)
            ones_sb = misc_pool.tile([P, 1], BF16, tag="ones")
            nc.vector.memset(ones_sb[:], 1.0)

            v_dram = dram_pool.tile([T, H], BF16)

            # ---- Phase A1: kT = (Wk xk)  resident in SBUF as [128, HO, T]
            kt_sb = kt_pool.tile([P, HO, T], BF16)
            wk_sb = wkv_pool.tile([P, EO, H], BF16, tag="w")
            nc.sync.dma_start(wk_sb[:], et(wk))
            for tb in range(NBLK):
                xk_sb = x_pool.tile([P, EO, BLK], BF16, tag="x")
                nc.sync.dma_start(xk_sb[:], et(xk)[:, :, tb * BLK : (tb + 1) * BLK])
                for ho in range(HO):
                    ps = ps_a.tile([P, BLK], F32, tag="ps_a")
                    for eo in range(EO):
                        nc.tensor.matmul(
                            ps[:],
                            wk_sb[:, eo, ho * P : (ho + 1) * P],
                            xk_sb[:, eo, :],
                            start=(eo == 0),
                            stop=(eo == EO - 1),
                        )
                    nc.vector.tensor_copy(
                        kt_sb[:, ho, tb * BLK : (tb + 1) * BLK], ps[:]
                    )

            # ---- Phase A2: v = (xv^T Wv) -> DRAM scratch [T, H] bf16
            wv_sb = wkv_pool.tile([P, EO, H], BF16, tag="w")
            nc.sync.dma_start(wv_sb[:], et(wv))
            # preload wq early so phase B doesn't stall on it
            wq_sb = wq_pool.tile([P, EO, H], BF16, tag="wq")
            nc.sync.dma_start(wq_sb[:], et(wq))
            for tt in range(TKT):
                xv_sb = x_pool.tile([P, EO, P], BF16, tag="xv")
                nc.sync.dma_start(xv_sb[:], et(xv)[:, :, tt * P : (tt + 1) * P])
                v_sb = vproj_pool.tile([P, H], BF16, tag="vproj")
                for hb in range(NBLK):
                    ps = ps_a.tile([P, BLK], F32, tag="ps_a")
                    for eo in range(EO):
                        nc.tensor.matmul(
                            ps[:],
                            xv_sb[:, eo, :],
                            wv_sb[:, eo, hb * BLK : (hb + 1) * BLK],
                            start=(eo == 0),
                            stop=(eo == EO - 1),
                        )
                    nc.vector.tensor_copy(v_sb[:, hb * BLK : (hb + 1) * BLK], ps[:])
                nc.sync.dma_start(v_dram[tt * P : (tt + 1) * P, :], v_sb[:])

            # ---- Phase B: per tq block of 512
            for j in range(NBLK):
                ntk = min(4 * j + 5, TKT)  # tk tiles (mask kidx <= qidx+1)

                xq_sb = x_pool.tile([P, EO, BLK], BF16, tag="x")
                nc.sync.dma_start(xq_sb[:], et(xq)[:, :, j * BLK : (j + 1) * BLK])

                # qT block [128, HO, 512]
                qt_sb = qt_pool.tile([P, HO, BLK], BF16)
                for ho in range(HO):
                    ps = ps_a.tile([P, BLK], F32, tag="ps_a")
                    for eo in range(EO):
                        nc.tensor.matmul(
                            ps[:],
                            wq_sb[:, eo, ho * P : (ho + 1) * P],
                            xq_sb[:, eo, :],
                            start=(eo == 0),
                            stop=(eo == EO - 1),
                        )
                    nc.vector.tensor_copy(qt_sb[:, ho, :], ps[:])

                # S^T tiles -> exp -> mask -> P^T  [128, ntk, 512] bf16
                pt_sb = pt_pool.tile([P, 4 * NBLK, BLK], BF16)
                for t in range(ntk):
                    ps = ps_a.tile([P, BLK], F32, tag="ps_a")
                    for ho in range(HO):
                        nc.tensor.matmul(
                            ps[:],
                            kt_sb[:, ho, t * P : (t + 1) * P],
                            qt_sb[:, ho, :],
                            start=(ho == 0),
                            stop=(ho == HO - 1),
                        )
                    nc.scalar.activation(
                        pt_sb[:, t, :],
                        ps[:],
                        mybir.ActivationFunctionType.Exp,
                        scale=float(E) ** -0.5,
                    )
                    m = t - 4 * j
                    if m >= 0:  # partial tile: zero the disallowed region
                        nc.vector.tensor_tensor(
                            pt_sb[:, t, :],
                            pt_sb[:, t, :],
                            masks_sb[:, BLK - m * P : 2 * BLK - m * P],
                            mybir.AluOpType.mult,
                        )

                # denominator: den[tq] = sum_tk P^T ; ones-matmul, [128, 4]
                den_ps = ps_d.tile([P, NBLK], F32)
                for s in range(NBLK):
                    for t in range(ntk):
                        nc.tensor.matmul(
                            den_ps[:, s : s + 1],
                            pt_sb[:, t, s * P : (s + 1) * P],
                            ones_sb[:],
                            start=(t == 0),
                            stop=(t == ntk - 1),
                        )
                recip_sb = misc_pool.tile([P, NBLK], F32, tag=f"recip{j}")
                nc.vector.reciprocal(recip_sb[:], den_ps[:])

                # out[tq, h] = sum_tk P^T.T @ v.  v tiles of this h-block
                # stay resident so the four s-chains use ONE psum each and
                # every normalize overlaps the next chain on PE.
                for hb in range(NBLK):
                    v_tiles = []
                    for t in range(ntk):
                        v_sb = v_pool.tile(
                            [P, BLK], BF16, tag="vs", name=f"v_{j}_{hb}_{t}"
                        )
                        nc.sync.dma_start(
                            v_sb[:],
                            v_dram[t * P : (t + 1) * P, hb * BLK : (hb + 1) * BLK],
                        )
                        v_tiles.append(v_sb)
                    for s in range(NBLK):
                        o_ps = ps_o.tile(
                            [P, BLK], F32, tag="ps_o", name=f"o_ps_{j}_{hb}_{s}"
                        )
                        for t in range(ntk):
                            nc.tensor.matmul(
                                o_ps[:],
                                pt_sb[:, t, s * P : (s + 1) * P],
                                v_tiles[t][:],
                                start=(t == 0),
                                stop=(t == ntk - 1),
                            )
                        o_sb = out_pool.tile([P, BLK], F32, tag="o")
                        nc.vector.tensor_scalar_mul(
                            o_sb[:], o_ps[:], recip_sb[:, s : s + 1]
                        )
                        nc.sync.dma_start(
                            out[
                                j * BLK + s * P : j * BLK + (s + 1) * P,
                                hb * BLK : (hb + 1) * BLK,
                            ],
                            o_sb[:],
                        )
    return nc


_DMA_TYPES = ("InstDMACopy", "InstTensorLoad", "InstTensorSave", "InstCollective")


def _split_waits(nc, limit=1):
    """This walrus build accepts only one sync-wait per TPB instruction.
    Move excess waits onto same-engine nops inserted just before the
    instruction (engine sequencers execute in order, so this is
    semantically identical)."""
    k = 0
    for f in nc.m.functions:
        for blk in f.blocks:
            new = []
            for inst in blk.instructions:
                si = inst.sync_info
                waits = list(si.on_wait) if si and si.on_wait else []
                if len(waits) > limit:
                    for w in waits[:-limit]:
                        nop = mybir.InstNoOp(name=f"wsplit-{k}", ins=[], outs=[])
                        k += 1
                        nop.engine = inst.engine
                        nop.sync_info = mybir.SyncInfo(on_wait=[w], on_update=[])
                        new.append(nop)
                    si.on_wait = waits[-limit:]
                new.append(inst)
            blk.instructions[:] = new
    return nc


_NC_CACHE = None


def _get_nc():
    global _NC_CACHE
    if _NC_CACHE is None:
        _NC_CACHE = _split_waits(_build())
    return _NC_CACHE


def _host_masks():
    # wide[p, c] = (p <= c - 511); slice [BLK-128m : 2*BLK-128m] yields the
    # partial-tile mask for diagonal offset m (p <= f - 128m + 1).
    p = np.arange(P)[:, None]
    c = np.arange(2 * BLK)[None, :]
    return (p <= c - (BLK - 1)).astype(ml_dtypes.bfloat16)


def kernel(key, query, value, Wk, Wq, Wv):
    bf = ml_dtypes.bfloat16
    wq_t = np.ascontiguousarray(Wq.T).astype(bf)  # [E, H]
    wk_t = np.ascontiguousarray(Wk.T).astype(bf)
    wv_t = np.ascontiguousarray(Wv.T).astype(bf)
    masks = _host_masks()

    in_maps = []
    for b in range(B):
        in_maps.append(
            {
                "xq": np.ascontiguousarray(query[b].T).astype(bf),
                "xk": np.ascontiguousarray(key[b].T).astype(bf),
                "xv": np.ascontiguousarray(value[b].T).astype(bf),
                "wq": wq_t,
                "wk": wk_t,
                "wv": wv_t,
                "masks": masks,
            }
        )

    nc = _get_nc()
    res = bass_utils.run_bass_kernel_spmd(nc, in_maps, core_ids=list(range(B)))
    return np.stack([res.results[i]["out"] for i in range(B)]).astype(np.float32)
